# revision 25
# baseline (speedup 1.0000x reference)
"""AdderNet BasicBlock (conv1x1 -> adder1x1 -> BN -> ReLU -> conv3x3 ->
adder3x3 -> BN -> ReLU -> +residual -> ReLU) on 8 Trainium2 NeuronCores.

Sharding: 8 cores = 4 images x 2 row-halves. Half-1 cores receive
vertically flipped inputs and row-flipped 3x3 weights so that every core
runs the IDENTICAL SPMD program ("top half of the image, zero-pad above,
real rows below"); the host flips their outputs back. Each core computes a
2-row halo of the intermediate layers redundantly; no inter-core
communication at all.

Per-core layout: channels (128) on SBUF partitions, spatial positions on
the free dimension. The adder (L1-distance) layers dominate: with
|d| = 2*relu(d) - d, each (co, tap) needs ONE fused relu(v - w) op
(DVE tensor_scalar(subtract, max) at 4x fp16 rate, or ACT Relu with
per-partition bias -w; co's are split across both engines), followed by a
cross-partition reduction matmul whose stationary matrix has a single
column (co%32) of 2.0 -- accumulated into PSUM rows [co] with 4
col-groups interleaved for PE sub-array concurrency. The "- sum_ci d"
part is 10 all-(-1) matmuls into the same accumulation; "+ sum_ci w"
folds into the BN bias on the host. BN+ReLU is one ACT op per layer
(scale = -gamma/sqrt(var+eps) also folds the adder negation).

The adder datapath runs in fp16 (values are O(1..100), so fp16's 11-bit
mantissa keeps the final error ~1e-3 relative); conv inputs/weights are
fp16 (PSUM accumulation is fp32), BN/residual/output are fp32.

All fp16 inputs are packed into ONE [128, 4756] host tensor (and the few
fp32 ones into another): TRN2 compute instructions can embed very few
sync waits (often just one), so each engine observes each input-DMA
semaphore once via a dummy read, and all real consumers ride single
data-dependency waits.
"""

import numpy as np

N_CORES = 8
C = 128
H = W = 28
HALF_H = 14  # output rows per core
XROWS = 16  # input rows per core (2-row halo below)
P1 = XROWS * W  # 448 positions for conv1/adder1
V2ROWS = 15  # conv2 output rows per core
P2 = V2ROWS * W  # 420
POUT = HALF_H * W  # 392
EPS = 1e-5

# fp16 packed-input column offsets
OFF_X = 0
OFF_W1 = OFF_X + P1  # 448
OFF_W2 = OFF_W1 + C
OFF_Z32 = OFF_W2 + 9 * C  # [C,64] strip: column 32 is 2.0, rest 0
OFF_NEG1 = OFF_Z32 + 64  # [C,128] of -1.0
NCOLS16 = OFF_NEG1 + C  # 1920

# fp32 packed-input column offsets (adder weight scalars must be fp32:
# the per-partition scalar operand of tensor_scalar/activation is read
# once per instruction and does not affect the 4x fp16 data rate)
OFF_XR = 0  # x rows 0..13 for the residual
OFF_S1 = OFF_XR + POUT
OFF_B1 = OFF_S1 + 1
OFF_S2 = OFF_B1 + 1
OFF_B2 = OFF_S2 + 1
OFF_WA1 = OFF_B2 + 1
OFF_WA1N = OFF_WA1 + C
OFF_WA2 = OFF_WA1N + C
OFF_WA2N = OFF_WA2 + 9 * C
NCOLS32 = OFF_WA2N + 9 * C  # 2956

# engine split: co with (co % 3 == ACT_MOD) go to the Scalar engine
ACT_MOD = 2

_CACHE = {}


def _build_nc():
    import concourse.bass as bass
    import concourse.tile as tile
    import concourse.mybir as mybir
    from concourse.tile import add_dep_helper

    f32 = mybir.dt.float32
    f16 = mybir.dt.float16
    Alu = mybir.AluOpType
    Act = mybir.ActivationFunctionType

    nc = bass.Bass(trn_type="TRN2")

    a16_d = nc.dram_tensor("a16", [C, NCOLS16], f16, kind="ExternalInput")
    a32_d = nc.dram_tensor("a32", [C, NCOLS32], f32, kind="ExternalInput")
    y_d = nc.dram_tensor("y", [C, HALF_H, W], f32, kind="ExternalOutput")

    with tile.TileContext(nc) as tc:
        with (
            tc.tile_pool(name="const", bufs=1) as const_pool,
            tc.tile_pool(name="work", bufs=1) as work_pool,
            tc.tile_pool(name="dv", bufs=40) as dv_pool,
            tc.tile_pool(name="da", bufs=20) as da_pool,
            tc.tile_pool(name="psum", bufs=1, space=bass.MemorySpace.PSUM) as psum_pool,
        ):
            a16 = const_pool.tile([C, NCOLS16], f16)
            in16 = nc.sync.dma_start(a16[:], a16_d[:])
            a32 = const_pool.tile([C, NCOLS32], f32)
            in32 = nc.sync.dma_start(a32[:], a32_d[:])

            # each engine observes both input-DMA semaphores once
            sink_t = const_pool.tile([C, 2], f32)
            nc.vector.tensor_copy(sink_t[:, 0:1], a16[:, 0:1])
            nc.vector.tensor_copy(sink_t[:, 1:2], a32[:, 0:1])
            sink2_t = const_pool.tile([C, 2], f32)
            nc.scalar.copy(sink2_t[:, 0:1], a16[:, 0:1])
            nc.scalar.copy(sink2_t[:, 1:2], a32[:, 0:1])

            x_v = a16[:, OFF_X : OFF_X + P1].rearrange("p (a b) -> p a b", a=XROWS)
            w1_v = a16[:, OFF_W1 : OFF_W1 + C]
            w2_v = a16[:, OFF_W2 : OFF_W2 + 9 * C].rearrange("p (t c) -> p t c", t=9)
            wa1_v = a32[:, OFF_WA1 : OFF_WA1 + C]
            wa1n_v = a32[:, OFF_WA1N : OFF_WA1N + C]
            wa2_v = a32[:, OFF_WA2 : OFF_WA2 + 9 * C].rearrange(
                "p (t c) -> p t c", t=9
            )
            wa2n_v = a32[:, OFF_WA2N : OFF_WA2N + 9 * C].rearrange(
                "p (t c) -> p t c", t=9
            )
            z32_v = a16[:, OFF_Z32 : OFF_Z32 + 64]
            neg1_v = a16[:, OFF_NEG1 : OFF_NEG1 + C]
            xr_v = a32[:, OFF_XR : OFF_XR + POUT].rearrange(
                "p (a b) -> p a b", a=HALF_H
            )
            s1_v = a32[:, OFF_S1 : OFF_S1 + 1]
            b1_v = a32[:, OFF_B1 : OFF_B1 + 1]
            s2_v = a32[:, OFF_S2 : OFF_S2 + 1]
            b2_v = a32[:, OFF_B2 : OFF_B2 + 1]

            # Per-engine relu(v - w) producers with the observed-tick pump
            # (slot-reuse WAR/WAW waits must collapse to one per inst).
            PUMP = 8
            prods = {"v": [], "a": []}

            def emit_d(win, w_col, wn_col, co, shape):
                eng = "a" if co % 3 == ACT_MOD else "v"
                lst = prods[eng]
                if lst and len(lst) % PUMP == 0:
                    if eng == "a":
                        dmy = nc.scalar.copy(sink2_t[:, 0:1], sink2_t[:, 0:1])
                    else:
                        dmy = nc.vector.memset(sink_t[:, 0:1], 0.0)
                    add_dep_helper(dmy.ins, lst[-1].ins, sync=True,
                                   reason="pump observed self-tick")
                pool = da_pool if eng == "a" else dv_pool
                d = pool.tile(shape, f16, tag="d" + eng, name="d" + eng)
                if eng == "a":
                    ins = nc.scalar.activation(
                        d[:], win, Act.Relu, bias=wn_col, scale=1.0,
                    )
                else:
                    ins = nc.vector.tensor_scalar(
                        d[:], win, w_col, 0.0,
                        op0=Alu.subtract, op1=Alu.max,
                    )
                lst.append(ins)
                return d

            # ---- layer 1: conv1 (1x1) ----
            # PSUM tiles are [C, 512] = one bank, so 32-row col-group block
            # offsets stay bank-aligned.
            v1_ps = psum_pool.tile([C, 512], f32)
            nc.tensor.matmul(v1_ps[:, 0:P1], w1_v, x_v, start=True, stop=True)
            v1_t = work_pool.tile([C, P1], f16)
            nc.vector.tensor_copy(v1_t[:], v1_ps[:, 0:P1])

            # ---- adder1 (1x1) ----
            # S1_ps[co,p] = 2*sum_ci relu(v-w) - sum_ci v  (+sum_ci w goes
            # into the host-folded BN bias)
            S1_ps = psum_pool.tile([C, 512], f32)
            nc.tensor.matmul(
                S1_ps[:, 0:P1], neg1_v, v1_t[:],
                start=True, stop=False, skip_group_check=True,
            )
            for c in range(32):
                for j in range(4):
                    co = 32 * j + c
                    d1 = emit_d(
                        v1_t[:], wa1_v[:, co : co + 1],
                        wa1n_v[:, co : co + 1], co, [C, P1],
                    )
                    nc.tensor.matmul(
                        S1_ps[32 * j : 32 * j + 32, 0:P1],
                        z32_v[:, 32 - c : 64 - c],
                        d1[:],
                        start=False,
                        stop=(c == 31),
                        tile_position=(0, 32 * j),
                        skip_group_check=True,
                    )

            # ---- u1 = Relu(S1*s1 + b1), into zero-padded u1_pad (fp16) ----
            u1_pad = work_pool.tile([C, 17, 30], f16)
            nc.vector.memset(u1_pad[:], 0.0)
            nc.scalar.activation(
                u1_pad[:, 1:17, 1:29],
                S1_ps[:, 0:P1].rearrange("p (a b) -> p a b", a=XROWS),
                Act.Relu, bias=b1_v, scale=s1_v,
            )

            # ---- conv2 (3x3, pad 1): 9 accumulating matmuls ----
            v2_ps = psum_pool.tile([C, 512], f32)
            for t in range(9):
                kh, kw = divmod(t, 3)
                nc.tensor.matmul(
                    v2_ps[:, 0:P2],
                    w2_v[:, t, :],
                    u1_pad[:, kh : kh + V2ROWS, kw : kw + W],
                    start=(t == 0),
                    stop=(t == 8),
                )
            v2_pad = work_pool.tile([C, 16, 30], f16)
            nc.vector.memset(v2_pad[:], 0.0)
            nc.vector.tensor_copy(
                v2_pad[:, 1:16, 1:29],
                v2_ps[:, 0:P2].rearrange("p (a b) -> p a b", a=V2ROWS),
            )

            # ---- adder2 (3x3, pad 1) ----
            S2_ps = psum_pool.tile([C, 512], f32)
            last_mms = []
            for t in range(9):
                kh, kw = divmod(t, 3)
                nc.tensor.matmul(
                    S2_ps[:, 0:POUT],
                    neg1_v,
                    v2_pad[:, kh : kh + HALF_H, kw : kw + W],
                    start=(t == 0), stop=False, skip_group_check=True,
                )
            for c in range(32):
                for t in range(9):
                    kh, kw = divmod(t, 3)
                    win = v2_pad[:, kh : kh + HALF_H, kw : kw + W]
                    for j in range(4):
                        co = 32 * j + c
                        d2 = emit_d(
                            win, wa2_v[:, t, co : co + 1],
                            wa2n_v[:, t, co : co + 1], co, [C, HALF_H, W],
                        )
                        mm = nc.tensor.matmul(
                            S2_ps[32 * j : 32 * j + 32, 0:POUT],
                            z32_v[:, 32 - c : 64 - c],
                            d2[:],
                            start=False,
                            stop=(c == 31 and t == 8),
                            tile_position=(0, 32 * j),
                            skip_group_check=True,
                        )
                        if c == 31 and t == 8:
                            last_mms.append(mm)

            # ---- out = Relu(Relu(S2*s2 + b2) + x) ----
            o2_t = work_pool.tile([C, HALF_H, W], f32)
            o2_ins = nc.scalar.activation(
                o2_t[:],
                S2_ps[:, 0:POUT].rearrange("p (a b) -> p a b", a=HALF_H),
                Act.Relu, bias=b2_v, scale=s2_v,
            )
            r_t = work_pool.tile([C, HALF_H, W], f32)
            nc.vector.tensor_add(r_t[:], o2_t[:], xr_v)
            y_t = work_pool.tile([C, HALF_H, W], f32)
            yrelu = nc.vector.tensor_scalar_max(y_t[:], r_t[:], 0.0)
            nc.sync.dma_start(y_d[:], y_t[:])
            # SP nops, each waiting on one outstanding proc: they advance
            # SP's observed clock so the kernel-tail Drain (CTRL_NO struct,
            # small embedded-wait budget) needs fewer waits of its own.
            for tgt in [in16, in32, o2_ins, yrelu] + last_mms:
                nop = nc.sync.nop(nofuse=True, hint="drain_prewait")
                add_dep_helper(nop.ins, tgt.ins, sync=True,
                               reason="drain: pre-observe proc tick on SP")

    return nc


def _shard_inputs(inputs):
    """Build the 8 per-core input dicts (flip trick for bottom halves)."""
    x = np.asarray(inputs["x"], np.float32)

    w_shift2 = np.asarray(inputs["w_shift2"], np.float32)
    w_add2 = np.asarray(inputs["w_add2"], np.float32)
    w_shift1 = np.asarray(inputs["w_shift1"], np.float32)
    w_add1 = np.asarray(inputs["w_add1"], np.float32)

    w1T = np.ascontiguousarray(w_shift1[:, :, 0, 0].T)  # [ci, co]
    wa1 = np.ascontiguousarray(w_add1[:, :, 0, 0].T)

    def prep2(ws2, wa2):
        # [co, ci, kh, kw] -> [ci, kh*kw, co] -> [ci, 9*co]
        w2T = ws2.reshape(C, C, 9).transpose(1, 2, 0).reshape(C, 9 * C)
        wa2T = wa2.reshape(C, C, 9).transpose(1, 2, 0).reshape(C, 9 * C)
        return w2T, wa2T

    w2T, wa2 = prep2(w_shift2, w_add2)
    w2Tf, wa2f = prep2(
        np.ascontiguousarray(w_shift2[:, :, ::-1, :]),
        np.ascontiguousarray(w_add2[:, :, ::-1, :]),
    )

    def bn_fold(g, beta, mean, var, wsum):
        # PSUM holds 2*sum relu(v-w) - sum v = S - wsum (S = sum |v-w|);
        # out = relu((-S)*inv + (beta - mean*inv))
        #     = relu(PSUM*(-inv) + (beta - mean*inv - wsum*inv))
        inv = np.asarray(g, np.float64) / np.sqrt(np.asarray(var, np.float64) + EPS)
        s = (-inv).astype(np.float32).reshape(C, 1)
        b = (
            np.asarray(beta, np.float64)
            - np.asarray(mean, np.float64) * inv
            - np.asarray(wsum, np.float64) * inv
        )
        return s, b.astype(np.float32).reshape(C, 1)

    # the on-device sums use fp16-rounded weights, so wsum must use the
    # SAME rounded values for |d| = 2 relu(d) - d to hold exactly
    wa1_16 = wa1.astype(np.float16)
    wa2_16 = wa2.astype(np.float16)
    wa2f_16 = wa2f.astype(np.float16)
    wsum1 = wa1_16.astype(np.float64).sum(axis=0)  # [co], sum over ci
    wsum2 = wa2_16.astype(np.float64).reshape(C, 9, C).sum(axis=(0, 1))  # [co]

    s1, b1 = bn_fold(
        inputs["bn1_gamma"], inputs["bn1_beta"], inputs["bn1_mean"],
        inputs["bn1_var"], wsum1,
    )
    s2, b2 = bn_fold(
        inputs["bn2_gamma"], inputs["bn2_beta"], inputs["bn2_mean"],
        inputs["bn2_var"], wsum2,
    )

    z32 = np.zeros((C, 64), np.float16)
    z32[:, 32] = 2.0
    neg1 = np.full((C, C), -1.0, np.float16)

    in_maps = []
    for k in range(N_CORES):
        n, half = divmod(k, 2)
        if half == 0:
            x_ext = x[n, :, 0:XROWS, :].reshape(C, P1)
            m_w2T, m_wa2 = w2T, wa2_16
        else:
            xf = x[n, :, ::-1, :]
            x_ext = np.ascontiguousarray(xf[:, 0:XROWS, :]).reshape(C, P1)
            m_w2T, m_wa2 = w2Tf, wa2f_16
        a16 = np.concatenate(
            [
                x_ext.astype(np.float16),
                w1T.astype(np.float16),
                m_w2T.astype(np.float16),
                z32,
                neg1,
            ],
            axis=1,
        )
        assert a16.shape == (C, NCOLS16), a16.shape
        wa2_32 = m_wa2.astype(np.float32)
        a32 = np.concatenate(
            [
                x_ext[:, 0:POUT].astype(np.float32),
                s1, b1, s2, b2,
                wa1_16.astype(np.float32),
                -wa1_16.astype(np.float32),
                wa2_32,
                -wa2_32,
            ],
            axis=1,
        )
        assert a32.shape == (C, NCOLS32), a32.shape
        in_maps.append(
            {
                "a16": np.ascontiguousarray(a16),
                "a32": np.ascontiguousarray(a32),
            }
        )
    return in_maps


def _gather_outputs(results):
    y = np.empty((4, C, H, W), np.float32)
    for k in range(N_CORES):
        n, half = divmod(k, 2)
        out = results[k]["y"]
        if half == 0:
            y[n, :, 0:HALF_H, :] = out
        else:
            y[n, :, HALF_H:H, :] = out[:, ::-1, :]
    return y


def kernel(_trace=False, **inputs):
    from concourse.bass_utils import run_bass_kernel_spmd

    if "nc" not in _CACHE:
        _CACHE["nc"] = _build_nc()
    nc = _CACHE["nc"]
    in_maps = _shard_inputs(inputs)
    res = run_bass_kernel_spmd(
        nc, in_maps, core_ids=list(range(N_CORES)), trace=_trace
    )
    out = _gather_outputs(res.results)
    if _trace:
        return out, res
    return out
